# revision 32
# baseline (speedup 1.0000x reference)
"""Trainium2 Bass kernel for nn_MultiHeadAttention (channel-attention transformer block).

Math (per batch b, with X* = reshape(*, [C, P]), P = 4096, C = 128, D = 512):
  Q = Xq @ (Wq/temp)^T, K = Xk @ Wk^T, V = Xv @ Wv^T            [C, D]
  per head h (8 heads, ld=64): A_h = softmax(Q_h K_h^T); O_h = A_h V_h
  O = silu(O); O = (O - mean)/(unbiased_std + eps)   (LN affine folded into fc)
  out_pre = (v + Wfc@ln_beta) + O @ (Wfc*ln_gamma)^T
  out = BatchNorm2d(out_pre)   (batch stats over (b,h,w), biased var)

Sharding: data-parallel over batch, 2 batches per core on 8 cores; BatchNorm
statistics combined with a tiny AllReduce ([128,2] per core), preceded by a
dummy warm-up AllReduce at kernel start that absorbs the CC rendezvous cost.

All tensors bf16 on the wire; PSUM accumulation and statistics in f32.
Attention computes S^T per head so the exp() output in SBUF is directly the
AV stationary operand (no attention-probability transposes); the softmax
denominator comes from an extra N=1 matmul against a ones vector.  Output is
written bf16 and upcast on the host.  (BASS_PACKED_ATTN=1 selects a batched
attention variant that CoreSim accepts but this runtime rejects; off by
default.)
"""

import os

import numpy as np

import concourse.mybir as mybir
import concourse.tile as tile
from concourse import bacc
from concourse.bass_utils import run_bass_kernel_spmd
from concourse.masks import make_identity

# ---- problem constants (hardcoded per contract) ----
B, C, HH, WW = 16, 128, 64, 64
P = HH * WW           # 4096
NH, LD = 8, 64
D = NH * LD           # 512
N_CORES = 8
BPC = B // N_CORES    # 2 batches per core
NPC = 32              # 128-row contraction chunks over P
NPT = 8               # 512-col output tiles over P
LN_EPS = 1e-6
BN_EPS = 1e-5
F32 = mybir.dt.float32
BF16 = mybir.dt.bfloat16

MODE = "bf16"
# BASS_BN_LOCAL=1: per-core BN stats (no collective) -- approximation, for timing
BN_LOCAL = os.environ.get("BASS_BN_LOCAL", "0") == "1"
CC_WARM = os.environ.get("BASS_CC_WARM", "1") == "1"
V2_IO = os.environ.get("BASS_V2_IO", "0") == "1"
PACKED = os.environ.get("BASS_PACKED_ATTN", "0") == "1"

_BUILD_CACHE: dict = {}
LAST_RESULTS = None  # BassKernelResults of the most recent run (for profiling)


def _emit(ctx, nc, tc, io):
    AF = mybir.ActivationFunctionType
    ALU = mybir.AluOpType
    AX = mybir.AxisListType

    consts = ctx.enter_context(tc.tile_pool(name="consts", bufs=1))
    wpool = ctx.enter_context(tc.tile_pool(name="wpool", bufs=3))
    apool = ctx.enter_context(tc.tile_pool(name="apool", bufs=3))
    fcpool = ctx.enter_context(tc.tile_pool(name="fcpool", bufs=1))
    sb = ctx.enter_context(tc.tile_pool(name="sb", bufs=2))
    atp = ctx.enter_context(tc.tile_pool(name="atp", bufs=4))
    keep = ctx.enter_context(tc.tile_pool(name="keep", bufs=1))
    small = ctx.enter_context(tc.tile_pool(name="small", bufs=4))
    stat = ctx.enter_context(tc.tile_pool(name="stat", bufs=1))
    dram = ctx.enter_context(tc.tile_pool(name="dram", bufs=1, space="DRAM"))

    ident = consts.tile([128, 128], BF16, tag="ident", name="ident")
    ident_f = consts.tile([128, 128], F32, tag="identf", name="identf")
    make_identity(nc, ident_f)
    nc.vector.tensor_copy(out=ident, in_=ident_f)

    bng = consts.tile([128, 1], F32, tag="bng", name="bng")
    bnb = consts.tile([128, 1], F32, tag="bnb", name="bnb")
    epsbn = consts.tile([128, 1], F32, tag="epsbn", name="epsbn")
    nc.gpsimd.dma_start(out=bng, in_=io["bng"][:, :])
    nc.gpsimd.dma_start(out=bnb, in_=io["bnb"][:, :])
    nc.vector.memset(epsbn, BN_EPS)
    ones = consts.tile([128, 1], BF16, tag="ones", name="ones")
    nc.vector.memset(ones, 1.0)

    # warm up the collective path: a dummy AllReduce with no data deps runs
    # during phase A and absorbs the CC rendezvous/skew cost
    if CC_WARM and not BN_LOCAL:
        dumin = dram.tile([128, 1], F32, tag="dumin", name="dumin")
        dumout = dram.tile([128, 1], F32, tag="dumout", name="dumout")
        nc.gpsimd.dma_start(out=dumin[:, :], in_=epsbn)
        nc.gpsimd.collective_compute(
            "AllReduce", ALU.add, replica_groups=[list(range(N_CORES))],
            ins=[dumin.opt()], outs=[dumout.opt()])

    # ---- phase A: QKV projections (activation-stationary, [c, D] per batch) ----
    ctx_a = tc.tile_pool(name="ps_proj", bufs=1, space="PSUM")
    ps_proj = ctx_a.__enter__()
    ctx_t = tc.tile_pool(name="ps_t", bufs=2, space="PSUM")
    ps_t = ctx_t.__enter__()
    warm = ps_t.tile([128, 128], BF16, tag="stp", name="warm")
    nc.tensor.transpose(warm[:, :], ident[:, :], ident[:, :])
    Qp = [ps_proj.tile([128, 512], F32, tag=f"Qp{b}", name=f"Qp{b}") for b in range(BPC)]
    Kp = [ps_proj.tile([128, 512], F32, tag=f"Kp{b}", name=f"Kp{b}") for b in range(BPC)]
    Vp = [ps_proj.tile([128, 512], F32, tag=f"Vp{b}", name=f"Vp{b}") for b in range(BPC)]

    NDMA = NPC // 2  # 2 pc-chunks per DMA
    for g in range(NDMA):
        a3 = apool.tile([128, 2, 3 * 256], BF16, tag="a3", name="a3")
        w3 = wpool.tile([128, 2, 3 * 512], BF16, tag="w3", name="w3")
        nc.sync.dma_start(out=a3, in_=io["acts"][g])
        (nc.scalar if g % 2 == 0 else nc.gpsimd).dma_start(out=w3, in_=io["wqkv"][g])
        for i in range(2):
            pc = 2 * g + i
            st = pc == 0
            sp = pc == NPC - 1
            for t, dst in ((0, Qp), (1, Kp), (2, Vp)):
                for b in range(BPC):
                    nc.tensor.matmul(dst[b][:, :],
                                     a3[:, i, t * 256 + b * 128:t * 256 + (b + 1) * 128],
                                     w3[:, i, t * 512:(t + 1) * 512], start=st, stop=sp)

    # late streams: wfc (fc weights) split across sync/vector, veff on scalar/gpsimd
    wfcts = []
    for pt in range(NPT):
        wfct = fcpool.tile([128, 4, 512], BF16, tag=f"wfct{pt}", name=f"wfct{pt}")
        (nc.sync if pt % 2 == 0 else nc.scalar).dma_start(out=wfct, in_=io["wfc"][pt])
        wfcts.append(wfct)
    veffs = []
    for b in range(BPC):
        t = keep.tile([128, P], BF16, tag=f"veff{b}", name=f"veff{b}")
        nc.gpsimd.dma_start(out=t, in_=io["veff"][b, :, :])
        veffs.append(t)

    # ---- evacuate PSUM -> bf16 SBUF, transpose Q/K to [d, c] ----
    # V_sb rows: per (b, h) 65 cols = [V_h | 1] so AV matmul emits denominator too
    V_sb = keep.tile([128, BPC, NH, 65], BF16, tag="V_sb", name="V_sb")
    for b in range(BPC):
        for h in range(NH):
            nc.vector.memset(V_sb[:, b, h, 64:65], 1.0)
    QTs, KTs = [], []
    for b in range(BPC):
        Q_sb = sb.tile([128, D], BF16, tag="Q_sb", name="Q_sb")
        K_sb = sb.tile([128, D], BF16, tag="K_sb", name="K_sb")
        nc.vector.tensor_copy(out=Q_sb, in_=Qp[b][:, :])
        nc.scalar.copy(out=K_sb, in_=Kp[b][:, :])
        for h in range(NH):
            nc.scalar.copy(out=V_sb[:, b, h, 0:64],
                           in_=Vp[b][:, h * 64:(h + 1) * 64])
        QT_sb = keep.tile([128, D], BF16, tag=f"QT_sb{b}", name=f"QT_sb{b}")
        KT_sb = keep.tile([128, D], BF16, tag=f"KT_sb{b}", name=f"KT_sb{b}")
        for src, dst in ((Q_sb, QT_sb), (K_sb, KT_sb)):
            for dc in range(4):
                tp = ps_t.tile([128, 128], BF16, tag="stp", name="stp")
                nc.tensor.transpose(tp[:, :], src[:, dc * 128:(dc + 1) * 128], ident[:, :])
                nc.vector.tensor_copy(out=dst[:, dc * 128:(dc + 1) * 128], in_=tp[:, :])
        QTs.append(QT_sb)
        KTs.append(KT_sb)
    ctx_t.__exit__(None, None, None)
    ctx_a.__exit__(None, None, None)

    # attention/fc PSUM pools: Sp 2 + Od 1 + den 1 + stp 2 + O2 2 = 8 banks
    ctx_b = tc.tile_pool(name="ps_attn", bufs=2, space="PSUM")
    ps_b = ctx_b.__enter__()

    pcols = stat.tile([128, 32], F32, tag="pcols", name="pcols")
    out_sb = [keep.tile([128, P], BF16, tag=f"osb{b}", name=f"osb{b}")
              for b in range(BPC)]

    def attn_ST(b, interleave=None):
        """8 S^T matmuls into 2 packed PSUM tiles; optional interleaved PE work."""
        Sp = [ps_b.tile([128, 512], F32, tag="Sp", name=f"Sp{b}{t}") for t in range(2)]
        for h in range(NH):
            po = (h % 2) * 64
            dsl = h // 2
            nc.tensor.matmul(Sp[h // 4][:, (h % 4) * 128:(h % 4) * 128 + 128],
                             KTs[b][po:po + 64, dsl * 128:(dsl + 1) * 128],
                             QTs[b][po:po + 64, dsl * 128:(dsl + 1) * 128],
                             start=True, stop=True)
            if interleave is not None:
                interleave(h)
        return Sp

    def attn_tail(b, Sp):
        """exp -> AV+den -> normalize, returns Osc [c, D] f32."""
        ATp = [atp.tile([128, 512], BF16, tag="ATp", name=f"ATp{b}{t}") for t in range(2)]
        for t in range(2):
            nc.scalar.activation(out=ATp[t], in_=Sp[t][:, :], func=AF.Exp)
        Od = ps_b.tile([128, 512], F32, tag="Od", name=f"Od{b}", bufs=1)
        Osc = sb.tile([128, D], F32, tag="Osc", name="Osc")
        for h in range(NH):
            AT = ATp[h // 4][:, (h % 4) * 128:(h % 4) * 128 + 128]
            nc.tensor.matmul(Od[:, h * 64:(h + 1) * 64], AT,
                             V_sb[:, b, h, 0:64], start=True, stop=True)
            den = ps_b.tile([128, 1], F32, tag="den", name="den", bufs=1)
            nc.tensor.matmul(den[:, :], AT, ones[:, :], start=True, stop=True)
            rs = small.tile([128, 1], F32, tag="rs", name="rs")
            nc.vector.reciprocal(rs, den[:, :])
            nc.vector.tensor_scalar_mul(out=Osc[:, h * 64:(h + 1) * 64],
                                        in0=Od[:, h * 64:(h + 1) * 64],
                                        scalar1=rs)
        return Osc

    def silu_ln_xt(b, Osc):
        sg = sb.tile([128, D], F32, tag="sg", name="sg")
        nc.scalar.activation(out=sg, in_=Osc, func=AF.Sigmoid)
        Osw = sb.tile([128, D], F32, tag="Osw", name="Osw")
        nc.vector.tensor_mul(out=Osw, in0=Osc, in1=sg)
        st6 = small.tile([128, 6], F32, tag="st6", name="st6")
        nc.vector.bn_stats(out=st6, in_=Osw)
        mv = small.tile([128, 2], F32, tag="mv", name="mv")
        nc.vector.bn_aggr(out=mv, in_=st6)
        sd = small.tile([128, 1], F32, tag="sd", name="sd")
        nc.scalar.activation(out=sd, in_=mv[:, 1:2], func=AF.Sqrt, scale=float(D) / (D - 1))
        nc.vector.tensor_scalar_add(out=sd, in0=sd, scalar1=LN_EPS)
        rstd = small.tile([128, 1], F32, tag="rstd", name="rstd")
        nc.vector.reciprocal(rstd, sd)
        xhat = sb.tile([128, D], BF16, tag="xhat", name="xhat")
        nc.vector.tensor_scalar(out=xhat, in0=Osw, scalar1=mv[:, 0:1], scalar2=rstd,
                                op0=ALU.subtract, op1=ALU.mult)
        xT = sb.tile([128, D], BF16, tag="xT", name="xT")
        for dc in range(4):
            tp = ps_b.tile([128, 128], BF16, tag="stp", name="stp")
            nc.tensor.transpose(tp[:, :], xhat[:, dc * 128:(dc + 1) * 128], ident[:, :])
            nc.scalar.copy(out=xT[:, dc * 128:(dc + 1) * 128], in_=tp[:, :])
        return xT

    def fc_seg(b, pt, xT):
        O2 = ps_b.tile([128, 512], F32, tag="O2", name="O2")
        for dc in range(4):
            nc.tensor.matmul(O2[:, :], xT[:, dc * 128:(dc + 1) * 128],
                             wfcts[pt][:, dc, :], start=dc == 0, stop=dc == 3)
        seg = out_sb[b][:, pt * 512:(pt + 1) * 512]
        col = b * NPT + pt
        nc.vector.tensor_add(out=seg, in0=O2[:, :],
                             in1=veffs[b][:, pt * 512:(pt + 1) * 512])
        nc.vector.reduce_sum(pcols[:, col:col + 1], seg, axis=AX.X)
        junk = sb.tile([128, 512], BF16, tag="junk", name="junk")
        nc.scalar.activation(out=junk, in_=seg, func=AF.Square,
                             accum_out=pcols[:, 16 + col:17 + col])

    def attn_v1(b):
        """per-head chain: S^T -> exp -> AV + den -> normalize."""
        Osc = sb.tile([128, D], F32, tag="Osc", name="Osc")
        for h in range(NH):
            po = (h % 2) * 64
            dsl = h // 2
            S = ps_b.tile([128, 128], F32, tag="Sp", name="S")
            nc.tensor.matmul(S[:, :], KTs[b][po:po + 64, dsl * 128:(dsl + 1) * 128],
                             QTs[b][po:po + 64, dsl * 128:(dsl + 1) * 128],
                             start=True, stop=True)
            AT = atp.tile([128, 128], BF16, tag="ATs", name="ATs")
            nc.scalar.activation(out=AT, in_=S[:, :], func=AF.Exp)
            Od = ps_b.tile([128, 65], F32, tag="Od", name="Od")
            nc.tensor.matmul(Od[:, :], AT[:, :], V_sb[:, b, h, :],
                             start=True, stop=True)
            rs = small.tile([128, 1], F32, tag="rs", name="rs")
            nc.vector.reciprocal(rs, Od[:, 64:65])
            nc.vector.tensor_scalar_mul(out=Osc[:, h * 64:(h + 1) * 64],
                                        in0=Od[:, 0:64], scalar1=rs)
        return Osc

    if PACKED:
        # batch 0: attention -> silu/LN -> (fc b0 interleaved with attention b1)
        Sp0 = attn_ST(0)
        Osc0 = attn_tail(0, Sp0)
        xT0 = silu_ln_xt(0, Osc0)
        Sp1 = attn_ST(1, interleave=lambda h: fc_seg(0, h, xT0))
        Osc1 = attn_tail(1, Sp1)
        xT1 = silu_ln_xt(1, Osc1)
        for pt in range(NPT):
            fc_seg(1, pt, xT1)
    else:
        xT0 = silu_ln_xt(0, attn_v1(0))
        for pt in range(NPT):
            fc_seg(0, pt, xT0)
        xT1 = silu_ln_xt(1, attn_v1(1))
        for pt in range(NPT):
            fc_seg(1, pt, xT1)

    # ---- BN stats AllReduce + normalize + store ----
    stats2 = stat.tile([128, 2], F32, tag="stats2", name="stats2")
    nc.vector.reduce_sum(stats2[:, 0:1], pcols[:, 0:16], axis=AX.X)
    nc.vector.reduce_sum(stats2[:, 1:2], pcols[:, 16:32], axis=AX.X)

    if BN_LOCAL:
        red = stats2
        inv_n = 1.0 / float(BPC * P)
    else:
        cin = dram.tile([128, 2], F32, tag="cin", name="cin")
        cout = dram.tile([128, 2], F32, tag="cout", name="cout")
        nc.gpsimd.dma_start(out=cin[:, :], in_=stats2)
        nc.gpsimd.collective_compute(
            "AllReduce",
            ALU.add,
            replica_groups=[list(range(N_CORES))],
            ins=[cin.opt()],
            outs=[cout.opt()],
        )
        red = stat.tile([128, 2], F32, tag="red", name="red")
        nc.gpsimd.dma_start(out=red[:, :], in_=cout[:, :])
        inv_n = 1.0 / float(B * P)

    mean = small.tile([128, 1], F32, tag="mean", name="mean")
    nc.scalar.mul(out=mean, in_=red[:, 0:1], mul=inv_n)
    ex2 = small.tile([128, 1], F32, tag="ex2", name="ex2")
    nc.scalar.mul(out=ex2, in_=red[:, 1:2], mul=inv_n)
    msq = small.tile([128, 1], F32, tag="msq", name="msq")
    nc.vector.tensor_mul(out=msq, in0=mean, in1=mean)
    var = small.tile([128, 1], F32, tag="var", name="var")
    nc.vector.tensor_sub(out=var, in0=ex2, in1=msq)
    sdv = small.tile([128, 1], F32, tag="sdv", name="sdv")
    nc.scalar.activation(out=sdv, in_=var, func=AF.Sqrt, bias=epsbn)
    invs = small.tile([128, 1], F32, tag="invs", name="invs")
    nc.vector.reciprocal(invs, sdv)
    scl = small.tile([128, 1], F32, tag="scl", name="scl")
    nc.vector.tensor_mul(out=scl, in0=bng, in1=invs)
    tmp = small.tile([128, 1], F32, tag="tmp", name="tmp")
    nc.vector.tensor_mul(out=tmp, in0=mean, in1=scl)
    shf = small.tile([128, 1], F32, tag="shf", name="shf")
    nc.vector.tensor_sub(out=shf, in0=bnb, in1=tmp)

    for b in range(BPC):
        for half in range(2):
            seg = out_sb[b][:, half * 2048:(half + 1) * 2048]
            nc.vector.tensor_scalar(out=seg, in0=seg, scalar1=scl, scalar2=shf,
                                    op0=ALU.mult, op1=ALU.add)
            nc.sync.dma_start(
                out=io["out"][b, :, half * 2048:(half + 1) * 2048], in_=seg)


def _build():
    key = (MODE, BN_LOCAL, CC_WARM, V2_IO, PACKED)
    if key in _BUILD_CACHE:
        return _BUILD_CACHE[key]
    nc = bacc.Bacc("TRN2", target_bir_lowering=False, debug=False, num_devices=N_CORES)
    io = {
        # [g, p(128), pc-in-pair, (q|k|v) x (b,c)]
        "acts": nc.dram_tensor("acts", [16, 128, 2, 768], BF16, kind="ExternalInput").ap(),
        # [g, p(128), pc-in-pair, (wq|wk|wv) x d]
        "wqkv": nc.dram_tensor("wqkv", [16, 128, 2, 1536], BF16, kind="ExternalInput").ap(),
        "veff": nc.dram_tensor("veff", [BPC, C, P], BF16, kind="ExternalInput").ap(),
        # [pt, d-in-chunk(128), dc, p-col(512)]
        "wfc": nc.dram_tensor("wfc", [NPT, 128, 4, 512], BF16, kind="ExternalInput").ap(),
        "bng": nc.dram_tensor("bng", [C, 1], F32, kind="ExternalInput").ap(),
        "bnb": nc.dram_tensor("bnb", [C, 1], F32, kind="ExternalInput").ap(),
        "out": nc.dram_tensor("out", [BPC, C, P], BF16, kind="ExternalOutput").ap(),
    }
    from contextlib import ExitStack
    with tile.TileContext(nc) as tc, ExitStack() as ctx:
        _emit(ctx, nc, tc, io)
    nc.compile()
    _BUILD_CACHE[key] = nc
    return nc


def _bf16(x):
    import ml_dtypes
    return np.ascontiguousarray(np.asarray(x, np.float32).astype(ml_dtypes.bfloat16))


def kernel(v, k, q, w_qs, w_ks, w_vs, w_fc, ln_gamma, ln_beta, temperature,
           bn_gamma, bn_beta, **_ignored):
    v = np.asarray(v, np.float32)
    k = np.asarray(k, np.float32)
    q = np.asarray(q, np.float32)
    w_qs = np.asarray(w_qs, np.float32)
    w_ks = np.asarray(w_ks, np.float32)
    w_vs = np.asarray(w_vs, np.float32)
    w_fc = np.asarray(w_fc, np.float32)
    ln_gamma = np.asarray(ln_gamma, np.float32)
    ln_beta = np.asarray(ln_beta, np.float32)
    temp = float(np.asarray(temperature))
    bn_gamma = np.asarray(bn_gamma, np.float32)
    bn_beta = np.asarray(bn_beta, np.float32)

    qf = q.reshape(B, C, P)
    kf = k.reshape(B, C, P)
    vf = v.reshape(B, C, P)

    # acts pack: [core, g, p, i, t, b, c] <- X_t[2*core+b, c, (2g+i)*128+p]
    A = np.stack([qf, kf, vf])                    # [3, B, C, P]
    A = A.reshape(3, N_CORES, BPC, C, 16, 2, 128)  # [t, core, b, c, g, i, p]
    A = A.transpose(1, 4, 6, 5, 0, 2, 3)           # [core, g, p, i, t, b, c]
    acts = _bf16(A.reshape(N_CORES, 16, 128, 2, 768))

    # wqkv pack: [g, p, i, t, d] <- W_t[d, (2g+i)*128+p]
    W3 = np.stack([w_qs / temp, w_ks, w_vs])       # [3, D, P]
    W3 = W3.reshape(3, D, 16, 2, 128).transpose(2, 4, 3, 0, 1)  # [g, p, i, t, d]
    wqkv = _bf16(W3.reshape(16, 128, 2, 1536))

    # wfc pack [pt, d, dc, p-col]: wfcT_eff[dc*128+d, pt*512+p]
    wfcT_eff = (w_fc * ln_gamma[None, :]).T        # [D, P]
    wfc = _bf16(wfcT_eff.reshape(4, 128, NPT, 512).transpose(2, 1, 0, 3))
    bias_fc = (w_fc @ ln_beta).astype(np.float32)
    veff = _bf16(vf + bias_fc[None, None, :])
    bng = np.ascontiguousarray(bn_gamma.reshape(C, 1))
    bnb = np.ascontiguousarray(bn_beta.reshape(C, 1))

    nc = _build()
    in_maps = []
    for i in range(N_CORES):
        bs = slice(BPC * i, BPC * (i + 1))
        in_maps.append({
            "acts": acts[i], "wqkv": wqkv, "veff": veff[bs], "wfc": wfc,
            "bng": bng, "bnb": bnb,
        })
    res = run_bass_kernel_spmd(nc, in_maps, core_ids=list(range(N_CORES)))
    global LAST_RESULTS
    LAST_RESULTS = res
    out = np.concatenate([np.asarray(res.results[i]["out"]) for i in range(N_CORES)],
                         axis=0)
    return out.reshape(B, C, HH, WW).astype(np.float32)


# revision 34
# speedup vs baseline: 1.0783x; 1.0783x over previous
"""Trainium2 Bass kernel for nn_MultiHeadAttention (channel-attention transformer block).

Math (per batch b, with X* = reshape(*, [C, P]), P = 4096, C = 128, D = 512):
  Q = Xq @ (Wq/temp)^T, K = Xk @ Wk^T, V = Xv @ Wv^T            [C, D]
  per head h (8 heads, ld=64): A_h = softmax(Q_h K_h^T); O_h = A_h V_h
  O = silu(O); O = (O - mean)/(unbiased_std + eps)   (LN affine folded into fc)
  out_pre = (v + Wfc@ln_beta) + O @ (Wfc*ln_gamma)^T
  out = BatchNorm2d(out_pre)   (batch stats over (b,h,w), biased var)

Sharding: data-parallel over batch, 2 batches per core on 8 cores; BatchNorm
statistics combined with a tiny AllReduce ([128,2] per core), preceded by a
dummy warm-up AllReduce at kernel start that absorbs the CC rendezvous cost.

All tensors bf16 on the wire; PSUM accumulation and statistics in f32.
Attention computes S^T per head so the exp() output in SBUF is directly the
AV stationary operand (no attention-probability transposes); the softmax
denominator comes from an extra N=1 matmul against a ones vector.  Output is
written bf16 and upcast on the host.  (BASS_PACKED_ATTN=1 selects a batched
attention variant that CoreSim accepts but this runtime rejects; off by
default.)
"""

import os

import numpy as np

import concourse.mybir as mybir
import concourse.tile as tile
from concourse import bacc
from concourse.bass_utils import run_bass_kernel_spmd
from concourse.masks import make_identity

# ---- problem constants (hardcoded per contract) ----
B, C, HH, WW = 16, 128, 64, 64
P = HH * WW           # 4096
NH, LD = 8, 64
D = NH * LD           # 512
N_CORES = 8
BPC = B // N_CORES    # 2 batches per core
NPC = 32              # 128-row contraction chunks over P
NPT = 8               # 512-col output tiles over P
LN_EPS = 1e-6
BN_EPS = 1e-5
F32 = mybir.dt.float32
BF16 = mybir.dt.bfloat16

MODE = "bf16"
# BASS_BN_LOCAL=1: per-core BN stats (no collective) -- approximation, for timing
BN_LOCAL = os.environ.get("BASS_BN_LOCAL", "0") == "1"
CC_WARM = os.environ.get("BASS_CC_WARM", "1") == "1"
V2_IO = os.environ.get("BASS_V2_IO", "0") == "1"
PACKED = os.environ.get("BASS_PACKED_ATTN", "0") == "1"

_BUILD_CACHE: dict = {}
LAST_RESULTS = None  # BassKernelResults of the most recent run (for profiling)


def _emit(ctx, nc, tc, io):
    AF = mybir.ActivationFunctionType
    ALU = mybir.AluOpType
    AX = mybir.AxisListType

    consts = ctx.enter_context(tc.tile_pool(name="consts", bufs=1))
    wpool = ctx.enter_context(tc.tile_pool(name="wpool", bufs=3))
    apool = ctx.enter_context(tc.tile_pool(name="apool", bufs=3))
    fcpool = ctx.enter_context(tc.tile_pool(name="fcpool", bufs=1))
    sb = ctx.enter_context(tc.tile_pool(name="sb", bufs=2))
    atp = ctx.enter_context(tc.tile_pool(name="atp", bufs=4))
    keep = ctx.enter_context(tc.tile_pool(name="keep", bufs=1))
    small = ctx.enter_context(tc.tile_pool(name="small", bufs=4))
    stat = ctx.enter_context(tc.tile_pool(name="stat", bufs=1))
    dram = ctx.enter_context(tc.tile_pool(name="dram", bufs=1, space="DRAM"))

    ident = consts.tile([128, 128], BF16, tag="ident", name="ident")
    ident_f = consts.tile([128, 128], F32, tag="identf", name="identf")
    make_identity(nc, ident_f)
    nc.vector.tensor_copy(out=ident, in_=ident_f)

    bng = consts.tile([128, 1], F32, tag="bng", name="bng")
    bnb = consts.tile([128, 1], F32, tag="bnb", name="bnb")
    epsbn = consts.tile([128, 1], F32, tag="epsbn", name="epsbn")
    nc.gpsimd.dma_start(out=bng, in_=io["bng"][:, :])
    nc.gpsimd.dma_start(out=bnb, in_=io["bnb"][:, :])
    nc.vector.memset(epsbn, BN_EPS)
    ones = consts.tile([128, 1], BF16, tag="ones", name="ones")
    nc.vector.memset(ones, 1.0)

    # warm up the collective path: a dummy AllReduce with no data deps runs
    # during phase A and absorbs the CC rendezvous/skew cost
    if CC_WARM and not BN_LOCAL:
        dumin = dram.tile([128, 1], F32, tag="dumin", name="dumin")
        dumout = dram.tile([128, 1], F32, tag="dumout", name="dumout")
        nc.gpsimd.dma_start(out=dumin[:, :], in_=epsbn)
        nc.gpsimd.collective_compute(
            "AllReduce", ALU.add, replica_groups=[list(range(N_CORES))],
            ins=[dumin.opt()], outs=[dumout.opt()])

    # ---- phase A: QKV projections (activation-stationary, [c, D] per batch) ----
    ctx_a = tc.tile_pool(name="ps_proj", bufs=1, space="PSUM")
    ps_proj = ctx_a.__enter__()
    ctx_t = tc.tile_pool(name="ps_t", bufs=2, space="PSUM")
    ps_t = ctx_t.__enter__()
    warm = ps_t.tile([128, 128], BF16, tag="stp", name="warm")
    nc.tensor.transpose(warm[:, :], ident[:, :], ident[:, :])
    Qp = [ps_proj.tile([128, 512], F32, tag=f"Qp{b}", name=f"Qp{b}") for b in range(BPC)]
    Kp = [ps_proj.tile([128, 512], F32, tag=f"Kp{b}", name=f"Kp{b}") for b in range(BPC)]
    Vp = [ps_proj.tile([128, 512], F32, tag=f"Vp{b}", name=f"Vp{b}") for b in range(BPC)]

    NDMA = NPC // 2  # 2 pc-chunks per DMA
    for g in range(NDMA):
        a3 = apool.tile([128, 2, 3 * 256], BF16, tag="a3", name="a3")
        w3 = wpool.tile([128, 2, 3 * 512], BF16, tag="w3", name="w3")
        nc.sync.dma_start(out=a3, in_=io["acts"][g])
        (nc.scalar if g % 2 == 0 else nc.gpsimd).dma_start(out=w3, in_=io["wqkv"][g])
        for i in range(2):
            pc = 2 * g + i
            st = pc == 0
            sp = pc == NPC - 1
            for t, dst in ((0, Qp), (1, Kp), (2, Vp)):
                for b in range(BPC):
                    nc.tensor.matmul(dst[b][:, :],
                                     a3[:, i, t * 256 + b * 128:t * 256 + (b + 1) * 128],
                                     w3[:, i, t * 512:(t + 1) * 512], start=st, stop=sp)

    # late streams: wfc (fc weights) split across sync/vector, veff on scalar/gpsimd
    wfcts = []
    for pt in range(NPT):
        wfct = fcpool.tile([128, 4, 512], BF16, tag=f"wfct{pt}", name=f"wfct{pt}")
        (nc.sync if pt % 2 == 0 else nc.scalar).dma_start(out=wfct, in_=io["wfc"][pt])
        wfcts.append(wfct)
    veffs = []
    for b in range(BPC):
        t = keep.tile([128, P], BF16, tag=f"veff{b}", name=f"veff{b}")
        nc.gpsimd.dma_start(out=t, in_=io["veff"][b, :, :])
        veffs.append(t)

    # ---- evacuate PSUM -> bf16 SBUF, transpose Q/K to [d, c] ----
    # V_sb rows: per (b, h) 65 cols = [V_h | 1] so AV matmul emits denominator too
    V_sb = keep.tile([128, BPC, NH, 65], BF16, tag="V_sb", name="V_sb")
    for b in range(BPC):
        for h in range(NH):
            nc.vector.memset(V_sb[:, b, h, 64:65], 1.0)
    QTs, KTs = [], []
    for b in range(BPC):
        Q_sb = sb.tile([128, D], BF16, tag="Q_sb", name="Q_sb")
        K_sb = sb.tile([128, D], BF16, tag="K_sb", name="K_sb")
        nc.vector.tensor_copy(out=Q_sb, in_=Qp[b][:, :])
        nc.scalar.copy(out=K_sb, in_=Kp[b][:, :])
        for h in range(NH):
            nc.scalar.copy(out=V_sb[:, b, h, 0:64],
                           in_=Vp[b][:, h * 64:(h + 1) * 64])
        QT_sb = keep.tile([128, D], BF16, tag=f"QT_sb{b}", name=f"QT_sb{b}")
        KT_sb = keep.tile([128, D], BF16, tag=f"KT_sb{b}", name=f"KT_sb{b}")
        for src, dst in ((Q_sb, QT_sb), (K_sb, KT_sb)):
            for dc in range(4):
                tp = ps_t.tile([128, 128], BF16, tag="stp", name="stp")
                nc.tensor.transpose(tp[:, :], src[:, dc * 128:(dc + 1) * 128], ident[:, :])
                nc.vector.tensor_copy(out=dst[:, dc * 128:(dc + 1) * 128], in_=tp[:, :])
        QTs.append(QT_sb)
        KTs.append(KT_sb)
    ctx_t.__exit__(None, None, None)
    ctx_a.__exit__(None, None, None)

    # attention/fc PSUM pools: Sp 2 + Od 1 + den 1 + stp 2 + O2 2 = 8 banks
    ctx_b = tc.tile_pool(name="ps_attn", bufs=2, space="PSUM")
    ps_b = ctx_b.__enter__()

    pcols = stat.tile([128, 32], F32, tag="pcols", name="pcols")
    out_sb = [keep.tile([128, P], BF16, tag=f"osb{b}", name=f"osb{b}")
              for b in range(BPC)]

    def attn_ST(b, interleave=None):
        """8 S^T matmuls into 2 packed PSUM tiles; optional interleaved PE work."""
        Sp = [ps_b.tile([128, 512], F32, tag="Sp", name=f"Sp{b}{t}") for t in range(2)]
        for h in range(NH):
            po = (h % 2) * 64
            dsl = h // 2
            nc.tensor.matmul(Sp[h // 4][:, (h % 4) * 128:(h % 4) * 128 + 128],
                             KTs[b][po:po + 64, dsl * 128:(dsl + 1) * 128],
                             QTs[b][po:po + 64, dsl * 128:(dsl + 1) * 128],
                             start=True, stop=True)
            if interleave is not None:
                interleave(h)
        return Sp

    def attn_tail(b, Sp):
        """exp -> AV+den -> normalize, returns Osc [c, D] f32."""
        ATp = [atp.tile([128, 512], BF16, tag="ATp", name=f"ATp{b}{t}") for t in range(2)]
        for t in range(2):
            nc.scalar.activation(out=ATp[t], in_=Sp[t][:, :], func=AF.Exp)
        Od = ps_b.tile([128, 512], F32, tag="Od", name=f"Od{b}", bufs=1)
        Osc = sb.tile([128, D], F32, tag="Osc", name="Osc")
        for h in range(NH):
            AT = ATp[h // 4][:, (h % 4) * 128:(h % 4) * 128 + 128]
            nc.tensor.matmul(Od[:, h * 64:(h + 1) * 64], AT,
                             V_sb[:, b, h, 0:64], start=True, stop=True)
            den = ps_b.tile([128, 1], F32, tag="den", name="den", bufs=1)
            nc.tensor.matmul(den[:, :], AT, ones[:, :], start=True, stop=True)
            rs = small.tile([128, 1], F32, tag="rs", name="rs")
            nc.vector.reciprocal(rs, den[:, :])
            nc.vector.tensor_scalar_mul(out=Osc[:, h * 64:(h + 1) * 64],
                                        in0=Od[:, h * 64:(h + 1) * 64],
                                        scalar1=rs)
        return Osc

    def silu_ln_xt(b, Osc):
        sg = sb.tile([128, D], F32, tag="sg", name="sg")
        nc.scalar.activation(out=sg, in_=Osc, func=AF.Sigmoid)
        Osw = sb.tile([128, D], F32, tag="Osw", name="Osw")
        nc.vector.tensor_mul(out=Osw, in0=Osc, in1=sg)
        st6 = small.tile([128, 6], F32, tag="st6", name="st6")
        nc.vector.bn_stats(out=st6, in_=Osw)
        mv = small.tile([128, 2], F32, tag="mv", name="mv")
        nc.vector.bn_aggr(out=mv, in_=st6)
        sd = small.tile([128, 1], F32, tag="sd", name="sd")
        nc.scalar.activation(out=sd, in_=mv[:, 1:2], func=AF.Sqrt, scale=float(D) / (D - 1))
        nc.vector.tensor_scalar_add(out=sd, in0=sd, scalar1=LN_EPS)
        rstd = small.tile([128, 1], F32, tag="rstd", name="rstd")
        nc.vector.reciprocal(rstd, sd)
        xhat = sb.tile([128, D], BF16, tag="xhat", name="xhat")
        nc.vector.tensor_scalar(out=xhat, in0=Osw, scalar1=mv[:, 0:1], scalar2=rstd,
                                op0=ALU.subtract, op1=ALU.mult)
        xT = sb.tile([128, D], BF16, tag="xT", name="xT")
        for dc in range(4):
            tp = ps_b.tile([128, 128], BF16, tag="stp", name="stp")
            nc.tensor.transpose(tp[:, :], xhat[:, dc * 128:(dc + 1) * 128], ident[:, :])
            nc.scalar.copy(out=xT[:, dc * 128:(dc + 1) * 128], in_=tp[:, :])
        return xT

    def fc_seg(b, pt, xT):
        O2 = ps_b.tile([128, 512], F32, tag="O2", name="O2")
        for dc in range(4):
            nc.tensor.matmul(O2[:, :], xT[:, dc * 128:(dc + 1) * 128],
                             wfcts[pt][:, dc, :], start=dc == 0, stop=dc == 3)
        seg = out_sb[b][:, pt * 512:(pt + 1) * 512]
        col = b * NPT + pt
        nc.vector.tensor_add(out=seg, in0=O2[:, :],
                             in1=veffs[b][:, pt * 512:(pt + 1) * 512])
        nc.vector.reduce_sum(pcols[:, col:col + 1], seg, axis=AX.X)
        junk = sb.tile([128, 512], BF16, tag="junk", name="junk")
        nc.scalar.activation(out=junk, in_=seg, func=AF.Square,
                             accum_out=pcols[:, 16 + col:17 + col])

    def attn_v1(b):
        """per-head chain: S^T -> exp -> AV + den -> normalize."""
        Osc = sb.tile([128, D], F32, tag="Osc", name="Osc")
        for h in range(NH):
            po = (h % 2) * 64
            dsl = h // 2
            S = ps_b.tile([128, 128], F32, tag="Sp", name="S")
            nc.tensor.matmul(S[:, :], KTs[b][po:po + 64, dsl * 128:(dsl + 1) * 128],
                             QTs[b][po:po + 64, dsl * 128:(dsl + 1) * 128],
                             start=True, stop=True)
            AT = atp.tile([128, 128], BF16, tag="ATs", name="ATs")
            nc.scalar.activation(out=AT, in_=S[:, :], func=AF.Exp)
            Od = ps_b.tile([128, 65], F32, tag="Od", name="Od")
            nc.tensor.matmul(Od[:, :], AT[:, :], V_sb[:, b, h, :],
                             start=True, stop=True)
            rs = small.tile([128, 1], F32, tag="rs", name="rs")
            nc.vector.reciprocal(rs, Od[:, 64:65])
            nc.vector.tensor_scalar_mul(out=Osc[:, h * 64:(h + 1) * 64],
                                        in0=Od[:, 0:64], scalar1=rs)
        return Osc

    if PACKED:
        # batch 0: attention -> silu/LN -> (fc b0 interleaved with attention b1)
        Sp0 = attn_ST(0)
        Osc0 = attn_tail(0, Sp0)
        xT0 = silu_ln_xt(0, Osc0)
        Sp1 = attn_ST(1, interleave=lambda h: fc_seg(0, h, xT0))
        Osc1 = attn_tail(1, Sp1)
        xT1 = silu_ln_xt(1, Osc1)
        for pt in range(NPT):
            fc_seg(1, pt, xT1)
    else:
        xT0 = silu_ln_xt(0, attn_v1(0))
        for pt in range(NPT):
            fc_seg(0, pt, xT0)
        xT1 = silu_ln_xt(1, attn_v1(1))
        for pt in range(NPT):
            fc_seg(1, pt, xT1)

    # ---- BN stats AllReduce + normalize + store ----
    stats2 = stat.tile([128, 2], F32, tag="stats2", name="stats2")
    nc.vector.reduce_sum(stats2[:, 0:1], pcols[:, 0:16], axis=AX.X)
    nc.vector.reduce_sum(stats2[:, 1:2], pcols[:, 16:32], axis=AX.X)

    if BN_LOCAL:
        red = stats2
        inv_n = 1.0 / float(BPC * P)
    else:
        cin = dram.tile([128, 2], F32, tag="cin", name="cin")
        cout = dram.tile([128, 2], F32, tag="cout", name="cout")
        nc.gpsimd.dma_start(out=cin[:, :], in_=stats2)
        nc.gpsimd.collective_compute(
            "AllReduce",
            ALU.add,
            replica_groups=[list(range(N_CORES))],
            ins=[cin.opt()],
            outs=[cout.opt()],
        )
        red = stat.tile([128, 2], F32, tag="red", name="red")
        nc.gpsimd.dma_start(out=red[:, :], in_=cout[:, :])
        inv_n = 1.0 / float(B * P)

    mean = small.tile([128, 1], F32, tag="mean", name="mean")
    nc.scalar.mul(out=mean, in_=red[:, 0:1], mul=inv_n)
    ex2 = small.tile([128, 1], F32, tag="ex2", name="ex2")
    nc.scalar.mul(out=ex2, in_=red[:, 1:2], mul=inv_n)
    msq = small.tile([128, 1], F32, tag="msq", name="msq")
    nc.vector.tensor_mul(out=msq, in0=mean, in1=mean)
    var = small.tile([128, 1], F32, tag="var", name="var")
    nc.vector.tensor_sub(out=var, in0=ex2, in1=msq)
    sdv = small.tile([128, 1], F32, tag="sdv", name="sdv")
    nc.scalar.activation(out=sdv, in_=var, func=AF.Sqrt, bias=epsbn)
    invs = small.tile([128, 1], F32, tag="invs", name="invs")
    nc.vector.reciprocal(invs, sdv)
    scl = small.tile([128, 1], F32, tag="scl", name="scl")
    nc.vector.tensor_mul(out=scl, in0=bng, in1=invs)
    tmp = small.tile([128, 1], F32, tag="tmp", name="tmp")
    nc.vector.tensor_mul(out=tmp, in0=mean, in1=scl)
    shf = small.tile([128, 1], F32, tag="shf", name="shf")
    nc.vector.tensor_sub(out=shf, in0=bnb, in1=tmp)

    for b in range(BPC):
        for half in range(2):
            seg = out_sb[b][:, half * 2048:(half + 1) * 2048]
            nc.vector.tensor_scalar(out=seg, in0=seg, scalar1=scl, scalar2=shf,
                                    op0=ALU.mult, op1=ALU.add)
            nc.sync.dma_start(
                out=io["out"][b, :, half * 2048:(half + 1) * 2048], in_=seg)


def _build():
    key = (MODE, BN_LOCAL, CC_WARM, V2_IO, PACKED)
    if key in _BUILD_CACHE:
        return _BUILD_CACHE[key]
    nc = bacc.Bacc("TRN2", target_bir_lowering=False, debug=False, num_devices=N_CORES)
    io = {
        # [g, p(128), pc-in-pair, (q|k|v) x (b,c)]
        "acts": nc.dram_tensor("acts", [16, 128, 2, 768], BF16, kind="ExternalInput").ap(),
        # [g, p(128), pc-in-pair, (wq|wk|wv) x d]
        "wqkv": nc.dram_tensor("wqkv", [16, 128, 2, 1536], BF16, kind="ExternalInput").ap(),
        "veff": nc.dram_tensor("veff", [BPC, C, P], BF16, kind="ExternalInput").ap(),
        # [pt, d-in-chunk(128), dc, p-col(512)]
        "wfc": nc.dram_tensor("wfc", [NPT, 128, 4, 512], BF16, kind="ExternalInput").ap(),
        "bng": nc.dram_tensor("bng", [C, 1], F32, kind="ExternalInput").ap(),
        "bnb": nc.dram_tensor("bnb", [C, 1], F32, kind="ExternalInput").ap(),
        "out": nc.dram_tensor("out", [BPC, C, P], BF16, kind="ExternalOutput").ap(),
    }
    from contextlib import ExitStack
    with tile.TileContext(nc) as tc, ExitStack() as ctx:
        _emit(ctx, nc, tc, io)
    nc.compile()
    _BUILD_CACHE[key] = nc
    return nc


def _bf16(x):
    import ml_dtypes
    return np.ascontiguousarray(np.asarray(x, np.float32).astype(ml_dtypes.bfloat16))


def kernel(v, k, q, w_qs, w_ks, w_vs, w_fc, ln_gamma, ln_beta, temperature,
           bn_gamma, bn_beta, **_ignored):
    v = np.asarray(v, np.float32)
    k = np.asarray(k, np.float32)
    q = np.asarray(q, np.float32)
    w_qs = np.asarray(w_qs, np.float32)
    w_ks = np.asarray(w_ks, np.float32)
    w_vs = np.asarray(w_vs, np.float32)
    w_fc = np.asarray(w_fc, np.float32)
    ln_gamma = np.asarray(ln_gamma, np.float32)
    ln_beta = np.asarray(ln_beta, np.float32)
    temp = float(np.asarray(temperature))
    bn_gamma = np.asarray(bn_gamma, np.float32)
    bn_beta = np.asarray(bn_beta, np.float32)

    qf = q.reshape(B, C, P)
    kf = k.reshape(B, C, P)
    vf = v.reshape(B, C, P)

    # acts pack: [core, g, p, i, t, b, c] <- X_t[2*core+b, c, (2g+i)*128+p]
    A = np.stack([qf, kf, vf])                    # [3, B, C, P]
    A = A.reshape(3, N_CORES, BPC, C, 16, 2, 128)  # [t, core, b, c, g, i, p]
    A = A.transpose(1, 4, 6, 5, 0, 2, 3)           # [core, g, p, i, t, b, c]
    acts = _bf16(A.reshape(N_CORES, 16, 128, 2, 768))

    # wqkv pack: [g, p, i, t, d] <- W_t[d, (2g+i)*128+p]
    W3 = np.stack([w_qs / temp, w_ks, w_vs])       # [3, D, P]
    W3 = W3.reshape(3, D, 16, 2, 128).transpose(2, 4, 3, 0, 1)  # [g, p, i, t, d]
    wqkv = _bf16(W3.reshape(16, 128, 2, 1536))

    # wfc pack [pt, d, dc, p-col]: wfcT_eff[dc*128+d, pt*512+p]
    wfcT_eff = (w_fc * ln_gamma[None, :]).T        # [D, P]
    wfc = _bf16(wfcT_eff.reshape(4, 128, NPT, 512).transpose(2, 1, 0, 3))
    bias_fc = (w_fc @ ln_beta).astype(np.float32)
    veff = _bf16(vf + bias_fc[None, None, :])
    bng = np.ascontiguousarray(bn_gamma.reshape(C, 1))
    bnb = np.ascontiguousarray(bn_beta.reshape(C, 1))

    nc = _build()
    in_maps = []
    for i in range(N_CORES):
        bs = slice(BPC * i, BPC * (i + 1))
        in_maps.append({
            "acts": acts[i], "wqkv": wqkv, "veff": veff[bs], "wfc": wfc,
            "bng": bng, "bnb": bnb,
        })
    res = run_bass_kernel_spmd(nc, in_maps, core_ids=list(range(N_CORES)))
    global LAST_RESULTS
    LAST_RESULTS = res
    out = np.concatenate([np.asarray(res.results[i]["out"]) for i in range(N_CORES)],
                         axis=0)
    return out.reshape(B, C, HH, WW).astype(np.float32)
